# revision 9
# baseline (speedup 1.0000x reference)
"""Trainium2 Bass kernel for nn_CombinedLoss (deep-supervision CE + spectrum loss).

Data parallel over batch (B=512 -> 64 spectra per core x 8 cores).

Host prep (layout/indexing only): peaks are mask-compacted and sorted per
spectrum; for every 0.5-Da mass bin the host extracts the 4 (mass, intensity)
peak pairs starting at searchsorted(bin_edge) ("windows"), and selects per ion
the window of the bin floor(2*(theo-100.51)) using a host-side replica of the
theoretical-mass computation (used ONLY to choose gather windows; the device
recomputes theo in f32 and any mismatch just yields far peaks that are masked
out by the d < 0.5 window test).  Targets are pre-gathered into x[t,b,s] =
logits[t,b,s,tgt] and the CE mask/weights into wM = w_t*mask (layout prep).

Device (per core, partition p = 2*b + s_half):
  CE: exp(logits) on ACT (t=0..4 in fp8-e3m4, t=5 f32), se_t reductions on
  POOL+DVE (bf16), lse = Ln(se), ce = accum(wM*lse) - accum(wM*x).
  Spectrum: expected mass = (sum_v e^x*aa)/(sum_v e^x) -> PE matmul chain
  (b-half selectors -> transpose+duplicate -> cumsum with upper-tri ones) ->
  theo on ACT -> d = win_mass - theo -> windowed softmax / Huber / intensity
  sums in bf16 on DVE -> per-ion contributions -> partials.
  One activation table (natural_log_exp_and_others) loaded manually at t=0.

Output: per-partition partials [128,4] = (ce_num, mask_cnt, spec_num,
spec_cnt); host reduces across partitions+cores and combines.
"""

import os
import sys

import numpy as np
import ml_dtypes

for _p in ("/opt/trn_rl_repo", "/root/.axon_site/_ro/trn_rl_repo"):
    if os.path.isdir(_p) and _p not in sys.path:
        sys.path.append(_p)

T, B, S, V = 6, 512, 40, 28
N_PEAKS = 512
NCORES = 8
BS = B // NCORES          # 64 spectra per core
PROTON = 1.007276
WATER = 18.010565
CO = 27.994915
MASS_TOL = 0.5
TEMP = 0.1
HUB_D = 0.2
CE_W = 1.0
SPEC_W = 0.1

NRES = S - 2              # 38 residues
NI = S - 3                # 37 ions per family
P_IONS = 3 * NI           # 111 (+1 pad -> 112)
NPAIR = 56                # ion slots per partition (2 halves x 56 = 112)
WG = 4                    # window width (peaks per ion)
KBINS = 4096              # 0.5-Da mass bins from 100.0
BIG = 1.0e9

_cached = {}


def _build_program():
    import concourse.bass as bass
    import concourse.bacc as bacc
    import concourse.mybir as mybir
    import concourse.tile as tile
    from concourse.masks import make_upper_triangular

    dt = mybir.dt
    Alu = mybir.AluOpType
    Act = mybir.ActivationFunctionType
    AX = mybir.AxisListType

    nc = bacc.Bacc("TRN2", target_bir_lowering=False, debug=False,
                   num_devices=NCORES)

    lg5_d = nc.dram_tensor("lg5", [128, 20, V], dt.float32, kind="ExternalInput")
    p2_d = nc.dram_tensor("p2", [128, 512], dt.float32, kind="ExternalInput")
    lgce_d = nc.dram_tensor("lgce", [128, 5, 20, V], dt.float8e3,
                            kind="ExternalInput")
    wini_d = nc.dram_tensor("wini", [128, NPAIR, WG], dt.bfloat16,
                            kind="ExternalInput")
    out_d = nc.dram_tensor("partials", [128, 4], dt.float32, kind="ExternalOutput")

    f32 = dt.float32
    bf16 = dt.bfloat16

    with tile.TileContext(nc) as tc:
        with tc.tile_pool(name="main", bufs=1) as pool, \
             tc.tile_pool(name="ps", bufs=1, space="PSUM") as psp, \
             nc.allow_low_precision(reason="bf16 spectrum/CE partial sums validated vs reference"):

            # ---------------- input DMAs (serial transfer resource) --------
            lg5 = pool.tile([128, 20, V], f32, tag="lg5")
            nc.sync.dma_start(out=lg5[:], in_=lg5_d.ap())
            p2 = pool.tile([128, 512], f32, tag="p2")
            nc.sync.dma_start(out=p2[:], in_=p2_d.ap())
            lgce = pool.tile([128, 5, 20, V], dt.float8e3, tag="lgce")
            nc.sync.dma_start(out=lgce[:], in_=lgce_d.ap())
            wini = pool.tile([128, NPAIR, WG], bf16, tag="wini")
            nc.sync.dma_start(out=wini[:], in_=wini_d.ap())

            winm = p2[:, 0:224].rearrange("p (i w) -> p i w", w=WG)
            xce = p2[:, 224:344]           # [128, 120] target logits (t,q)
            wMce = p2[:, 344:464]          # [128, 120] w_t * mask
            tmf = p2[:, 464:484]           # [128, 20] raw target_mask
            aar = p2[:, 484:512]           # [128, 28] aa masses

            # ---------------- activation table: one combined load ----------
            ld = mybir.InstLoadActFuncSet(
                name=nc.get_next_instruction_name(), ins=[], outs=[])
            ld.act_func_set_id = 6     # natural_log_exp_and_others
            nc.scalar.add_instruction(ld)

            # ---------------- constant selectors (pre-input) ----------------
            iota_2b = pool.tile([128, 64], dt.int32, tag="iota_2b")
            nc.gpsimd.iota(iota_2b[:], pattern=[[2, 64]], channel_multiplier=0)
            iota_pp = pool.tile([128, 1], dt.int32, tag="iota_pp")
            nc.gpsimd.iota(iota_pp[:], pattern=[[0, 1]], channel_multiplier=1)
            i2b_f = pool.tile([128, 64], f32, tag="i2b_f")
            nc.vector.tensor_copy(out=i2b_f[:], in_=iota_2b[:])
            pp_f = pool.tile([128, 1], f32, tag="pp_f")
            nc.vector.tensor_copy(out=pp_f[:], in_=iota_pp[:])
            pm1_f = pool.tile([128, 1], f32, tag="pm1_f")
            nc.vector.tensor_scalar(out=pm1_f[:], in0=pp_f[:], scalar1=1.0,
                                    scalar2=None, op0=Alu.subtract)
            selh0 = pool.tile([128, 64], f32, tag="selh0")
            nc.vector.tensor_scalar(out=selh0[:], in0=i2b_f[:], scalar1=pp_f[:],
                                    scalar2=None, op0=Alu.is_equal)
            selh1 = pool.tile([128, 64], f32, tag="selh1")
            nc.vector.tensor_scalar(out=selh1[:], in0=i2b_f[:], scalar1=pm1_f[:],
                                    scalar2=None, op0=Alu.is_equal)
            iota_q = pool.tile([64, 2, 64], dt.int32, tag="iota_q")
            nc.gpsimd.iota(iota_q[:], pattern=[[0, 2], [1, 64]],
                           channel_multiplier=0)
            iq_f = pool.tile([64, 2, 64], f32, tag="iq_f")
            nc.vector.tensor_copy(out=iq_f[:], in_=iota_q[:])
            sel128 = pool.tile([64, 128], f32, tag="sel128")
            nc.vector.tensor_scalar(out=sel128[:],
                                    in0=iq_f[:].rearrange("p a b -> p (a b)"),
                                    scalar1=pp_f[0:64], scalar2=None,
                                    op0=Alu.is_equal)
            ut38 = pool.tile([NRES, NRES], f32, tag="ut38")
            make_upper_triangular(nc, ut38[:], val=1.0)
            b_pro = pool.tile([128, 1], f32, tag="b_pro")
            nc.vector.memset(b_pro[:], PROTON)
            b_wp = pool.tile([128, 1], f32, tag="b_wp")
            nc.vector.memset(b_wp[:], WATER + PROTON)
            b_pco = pool.tile([128, 1], f32, tag="b_pco")
            nc.vector.memset(b_pco[:], PROTON - CO)
            b_md = pool.tile([128, 1], f32, tag="b_md")
            nc.vector.memset(b_md[:], -HUB_D)

            # ---------------- t=5 spectrum head (critical path) -------------
            e5 = pool.tile([128, 20, V], f32, tag="e5")
            nc.scalar.activation(out=e5[:], in_=lg5[:], func=Act.Exp)
            se5 = pool.tile([128, 20], f32, tag="se5")
            nc.vector.tensor_reduce(out=se5[:], in_=e5[:], axis=AX.X, op=Alu.add)
            prod5 = pool.tile([128, 20, V], f32, tag="prod5")
            nc.gpsimd.tensor_tensor(out=prod5[:], in0=e5[:],
                                    in1=aar[:, None, :].broadcast_to([128, 20, V]),
                                    op=Alu.mult)
            nume = pool.tile([128, 20], f32, tag="nume")
            nc.vector.tensor_reduce(out=nume[:], in_=prod5[:], axis=AX.X,
                                    op=Alu.add)
            rec5 = pool.tile([128, 20], f32, tag="rec5")
            nc.vector.reciprocal_approx_fast(out=rec5[:], in_=se5[:])
            expected = pool.tile([128, 20], f32, tag="expected")
            nc.vector.tensor_tensor(out=expected[:], in0=nume[:], in1=rec5[:],
                                    op=Alu.mult)

            # exp64[b, s] for s=1..38 via half selectors (PE)
            exp64_ps = psp.tile([64, S], f32, tag="exp64_ps")
            nc.tensor.matmul(out=exp64_ps[:, 0:20], lhsT=selh0[:],
                             rhs=expected[:], start=True, stop=True)
            nc.tensor.matmul(out=exp64_ps[:, 20:40], lhsT=selh1[:],
                             rhs=expected[:], start=True, stop=True)
            exp64r = pool.tile([64, NRES], f32, tag="exp64r")
            nc.vector.tensor_copy(out=exp64r[:], in_=exp64_ps[:, 1:1 + NRES])

            # transpose + duplicate across halves: expTdup[r, p] = res[r, p%64]
            expTdup_ps = psp.tile([NRES, 128], f32, tag="expTdup_ps")
            nc.tensor.matmul(out=expTdup_ps[:], lhsT=exp64r[:], rhs=sel128[:],
                             start=True, stop=True)
            expTdup = pool.tile([NRES, 128], f32, tag="expTdup")
            nc.vector.tensor_copy(out=expTdup[:], in_=expTdup_ps[:])

            # cumsum over residues (both halves at once)
            cum_ps = psp.tile([128, NRES], f32, tag="cum_ps")
            nc.tensor.matmul(out=cum_ps[:], lhsT=expTdup[:], rhs=ut38[:],
                             start=True, stop=True)

            # theo families on ACT straight out of PSUM
            theoK = pool.tile([128, 112], f32, tag="theoK")
            nc.scalar.activation(out=theoK[:, 0:37], in_=cum_ps[:, 0:37],
                                 func=Act.Identity, bias=b_pro[:])
            nc.scalar.activation(out=theoK[:, 37:38], in_=cum_ps[:, 37:38],
                                 func=Act.Identity, bias=b_wp[:])
            nc.scalar.activation(out=theoK[:, 38:74], in_=cum_ps[:, 0:36],
                                 func=Act.Identity, scale=-1.0,
                                 bias=theoK[:, 37:38])
            nc.scalar.activation(out=theoK[:, 74:111], in_=cum_ps[:, 0:37],
                                 func=Act.Identity, bias=b_pco[:])
            nc.vector.memset(theoK[:, 111:112], -BIG)

            # stack even/odd ions onto partition halves
            theoV = theoK[:].rearrange("p (i two) -> p i two", two=2)
            theo_stk = pool.tile([128, NPAIR], f32, tag="theo_stk")
            nc.vector.tensor_copy(out=theo_stk[0:64], in_=theoV[0:64, :, 0])
            nc.vector.tensor_copy(out=theo_stk[64:128], in_=theoV[64:128, :, 1])

            # ---------------- CE exps (fp8 -> bf16) -------------------------
            se_all = pool.tile([128, 6, 20], bf16, tag="se_all")
            e04 = pool.tile([128, 5, 20, V], bf16, tag="e04")
            # chunked so ACT can interleave with the theo segments
            nc.scalar.activation(out=e04[:, 0:2], in_=lgce[:, 0:2], func=Act.Exp)
            nc.scalar.activation(out=e04[:, 2:4], in_=lgce[:, 2:4], func=Act.Exp)
            nc.scalar.activation(out=e04[:, 4:5], in_=lgce[:, 4:5], func=Act.Exp)
            nc.vector.tensor_copy(out=se_all[:, 5], in_=se5[:])

            partials = pool.tile([128, 4], f32, tag="partials")
            junkp = pool.tile([128, 120], f32, tag="junkp")
            ce2 = pool.tile([128, 1], f32, tag="ce2")
            nc.vector.scalar_tensor_tensor(out=junkp[:], in0=xce, scalar=1.0,
                                           in1=wMce, op0=Alu.mult, op1=Alu.mult,
                                           accum_out=ce2[:])
            nc.vector.tensor_reduce(out=partials[:, 1:2], in_=tmf, axis=AX.X,
                                    op=Alu.add)

            # ---------------- S3: windowed softmax (bf16) -------------------
            # se-reduce chunks are hand-interleaved into DVE wait gaps
            theoB = theo_stk[:, :, None].broadcast_to([128, NPAIR, WG])
            d0 = pool.tile([128, NPAIR, WG], f32, tag="d0")
            nc.vector.tensor_tensor(out=d0[:], in0=winm, in1=theoB,
                                    op=Alu.subtract)
            dd = pool.tile([128, NPAIR, WG], bf16, tag="dd")
            nc.vector.scalar_tensor_tensor(out=dd[:], in0=d0[:], scalar=-1.0,
                                           in1=d0[:], op0=Alu.mult, op1=Alu.max)
            nc.vector.tensor_reduce(out=se_all[:, 0:1], in_=e04[:, 0:1],
                                    axis=AX.X, op=Alu.add)
            ee = pool.tile([128, NPAIR, WG], bf16, tag="ee")
            nc.scalar.activation(out=ee[:], in_=dd[:], func=Act.Exp,
                                 scale=-1.0 / TEMP)
            ew = pool.tile([128, NPAIR, WG], bf16, tag="ew")
            nc.vector.scalar_tensor_tensor(out=ew[:], in0=dd[:], scalar=MASS_TOL,
                                           in1=ee[:], op0=Alu.is_lt, op1=Alu.mult)
            den = pool.tile([128, NPAIR], f32, tag="den")
            nc.vector.tensor_reduce(out=den[:], in_=ew[:], axis=AX.X, op=Alu.add)
            nc.vector.tensor_reduce(out=se_all[:, 1:2], in_=e04[:, 1:2],
                                    axis=AX.X, op=Alu.add)

            # huber via hub2 = x^2 - relu(x-delta)^2 (= 2*hub after host *0.5)
            xmin = pool.tile([128, NPAIR, WG], bf16, tag="xmin")
            nc.vector.tensor_scalar(out=xmin[:], in0=dd[:], scalar1=MASS_TOL,
                                    scalar2=None, op0=Alu.min)
            qsq = pool.tile([128, NPAIR, WG], bf16, tag="qsq")
            nc.scalar.activation(out=qsq[:], in_=xmin[:], func=Act.Square)
            rrel = pool.tile([128, NPAIR, WG], bf16, tag="rrel")
            nc.scalar.activation(out=rrel[:], in_=xmin[:], func=Act.Relu,
                                 bias=b_md[:])
            rsq = pool.tile([128, NPAIR, WG], bf16, tag="rsq")
            nc.scalar.activation(out=rsq[:], in_=rrel[:], func=Act.Square)

            # den-dependent S4 head (runs while ACT does the huber squares)
            nm = pool.tile([128, NPAIR], f32, tag="nm")
            nc.vector.tensor_scalar(out=nm[:], in0=den[:], scalar1=0.0,
                                    scalar2=None, op0=Alu.is_gt)
            nc.vector.tensor_reduce(out=partials[:, 3:4], in_=nm[:], axis=AX.X,
                                    op=Alu.add)
            dsafe = pool.tile([128, NPAIR], f32, tag="dsafe")
            nc.vector.tensor_scalar(out=dsafe[:], in0=den[:], scalar1=1e-6,
                                    scalar2=None, op0=Alu.max)
            rec = pool.tile([128, NPAIR], f32, tag="rec")
            nc.vector.reciprocal_approx_fast(out=rec[:], in_=dsafe[:])
            r2 = pool.tile([128, NPAIR], f32, tag="r2")
            nc.vector.tensor_tensor(out=r2[:], in0=rec[:], in1=rec[:],
                                    op=Alu.mult)
            nc.vector.tensor_reduce(out=se_all[:, 2:3], in_=e04[:, 2:3],
                                    axis=AX.X, op=Alu.add)

            hub2 = pool.tile([128, NPAIR, WG], bf16, tag="hub2")
            nc.vector.tensor_tensor(out=hub2[:], in0=qsq[:], in1=rsq[:],
                                    op=Alu.subtract)
            he = pool.tile([128, NPAIR, WG], bf16, tag="he")
            nc.gpsimd.tensor_tensor(out=he[:], in0=ew[:], in1=hub2[:], op=Alu.mult)
            ie = pool.tile([128, NPAIR, WG], bf16, tag="ie")
            nc.gpsimd.tensor_tensor(out=ie[:], in0=ew[:], in1=wini[:],
                                    op=Alu.mult)
            nc.vector.tensor_reduce(out=se_all[:, 3:4], in_=e04[:, 3:4],
                                    axis=AX.X, op=Alu.add)
            hubnum = pool.tile([128, NPAIR], bf16, tag="hubnum")
            nc.vector.tensor_reduce(out=hubnum[:], in_=he[:], axis=AX.X,
                                    op=Alu.add)
            iwnum = pool.tile([128, NPAIR], bf16, tag="iwnum")
            nc.vector.tensor_reduce(out=iwnum[:], in_=ie[:], axis=AX.X,
                                    op=Alu.add)
            nc.vector.tensor_reduce(out=se_all[:, 4:5], in_=e04[:, 4:5],
                                    axis=AX.X, op=Alu.add)

            # ---------------- S4 tail -------------------------------------
            t1 = pool.tile([128, NPAIR], f32, tag="t1")
            nc.vector.tensor_tensor(out=t1[:], in0=hubnum[:], in1=iwnum[:],
                                    op=Alu.mult)
            t2 = pool.tile([128, NPAIR], f32, tag="t2")
            nc.vector.tensor_tensor(out=t2[:], in0=t1[:], in1=r2[:], op=Alu.mult)
            junk56 = pool.tile([128, NPAIR], f32, tag="junk56")
            nc.vector.scalar_tensor_tensor(
                out=junk56[:], in0=t2[:], scalar=1.0, in1=t2[:],
                op0=Alu.mult, op1=Alu.max, accum_out=partials[:, 2:3])

            # ---------------- CE tail ---------------------------------------
            lse_all = pool.tile([128, 6, 20], bf16, tag="lse_all")
            nc.scalar.activation(out=lse_all[:].rearrange("p a b -> p (a b)"),
                                 in_=se_all[:].rearrange("p a b -> p (a b)"),
                                 func=Act.Ln)
            ce1 = pool.tile([128, 1], f32, tag="ce1")
            junk120 = pool.tile([128, 120], f32, tag="junk120")
            nc.vector.scalar_tensor_tensor(
                out=junk120[:], in0=lse_all[:].rearrange("p a b -> p (a b)"),
                scalar=1.0, in1=wMce, op0=Alu.mult, op1=Alu.mult,
                accum_out=ce1[:])
            nc.vector.scalar_tensor_tensor(out=partials[:, 0:1], in0=ce2[:],
                                           scalar=-1.0, in1=ce1[:],
                                           op0=Alu.mult, op1=Alu.add)

            # ---------------- output ----------------------------------------
            nc.sync.dma_start(out=out_d.ap(), in_=partials[:])

    nc.compile()
    return nc


def _get_nc():
    if "nc" not in _cached:
        _cached["nc"] = _build_program()
    return _cached["nc"]


def _part_pack(a):
    """[B, S, ...] -> per-core [128, 20, ...] with partition p = 2*b_loc + s//20."""
    shp = a.shape
    return a.reshape(B, 2, 20, *shp[2:])


def _host_prep(all_logits, targets, target_mask, observed_masses,
               observed_intensities, peak_mask, aa_masses):
    lg = np.asarray(all_logits, dtype=np.float32)
    tgt = np.asarray(targets, dtype=np.int64)
    tmask = np.asarray(target_mask)
    obs = np.asarray(observed_masses, dtype=np.float32)
    inten = np.asarray(observed_intensities, dtype=np.float32)
    pmask = np.asarray(peak_mask)
    aa = np.asarray(aa_masses, dtype=np.float32)

    # ---- peak compaction (masked -> BIG tail, order preserved) ----
    key = np.where(pmask, obs, np.inf)
    order = np.argsort(key, axis=-1, kind="stable")
    obs_eff = np.take_along_axis(np.where(pmask, obs, BIG).astype(np.float32),
                                 order, axis=-1)
    int_eff = np.take_along_axis(inten, order, axis=-1)
    obs_pad = np.concatenate(
        [obs_eff, np.full((B, WG), BIG, np.float32)], axis=1)
    int_pad = np.concatenate(
        [int_eff, np.zeros((B, WG), np.float32)], axis=1)

    # ---- host replica of theo (window selection only) ----
    m = lg[T - 1].max(axis=-1, keepdims=True)
    p5 = np.exp(lg[T - 1] - m)
    p5 /= p5.sum(axis=-1, keepdims=True)
    expc = p5 @ aa                                  # [B, S]
    res = expc[:, 1:1 + NRES]                       # [B, 38]
    cum = np.cumsum(res, axis=1)                    # [B, 38]
    b_i = cum[:, 0:NI] + PROTON                     # [B, 37]
    lastwp = cum[:, NRES - 1:NRES] + WATER + PROTON
    y_i = np.concatenate(
        [lastwp, lastwp - cum[:, 0:NI - 1]], axis=1)  # [B, 37]
    a_i = b_i - CO
    theo = np.concatenate(
        [b_i, y_i, a_i, np.full((B, 1), BIG, np.float32)], axis=1)  # [B,112]

    kbin = np.clip(np.floor(2.0 * (theo - 100.0 - MASS_TOL - 0.01)),
                   0, KBINS - 1).astype(np.int64)                   # [B,112]
    edges = 100.0 + 0.5 * kbin.astype(np.float32)
    # first peak >= edge per ion
    idx = np.empty((B, 112), dtype=np.int64)
    for b in range(B):
        idx[b] = np.searchsorted(obs_eff[b], edges[b])
    gi = idx[..., None] + np.arange(WG)[None, None, :]              # [B,112,4]
    gi2 = gi.reshape(B, 112 * WG)
    win_m = np.take_along_axis(obs_pad, gi2, axis=1).reshape(B, 112, WG)
    win_i = np.take_along_axis(int_pad, gi2, axis=1).reshape(B, 112, WG)

    # device stacked layout: [128, 56, WG], p = h*64 + b_loc holds ions 2i+h
    win_m = win_m.reshape(B, NPAIR, 2, WG)
    win_i = win_i.reshape(B, NPAIR, 2, WG)

    # ---- CE host layout prep ----
    x = np.take_along_axis(lg, tgt[None, :, :, None], axis=3)[..., 0]  # [T,B,S]
    Mm = (tmask & (tgt != 0)).astype(np.float32)                       # [B,S]
    w = (np.arange(1, T + 1, dtype=np.float32) / 21.0)
    # partition-packed views
    xp = _part_pack(x.transpose(1, 2, 0))          # [B,2,20,T]
    wMp = _part_pack(Mm)[..., None] * w            # [B,2,20,T]
    tmp = _part_pack(tmask.astype(np.float32))     # [B,2,20]
    lgp = _part_pack(lg.transpose(1, 2, 0, 3))     # [B,2,20,T,V]

    in_maps = []
    for c in range(NCORES):
        sl = slice(c * BS, (c + 1) * BS)
        lg_c = lgp[sl]                             # [64,2,20,T,V]
        lg5_c = np.ascontiguousarray(
            lg_c[:, :, :, T - 1].reshape(128, 20, V))
        lgce_c = np.ascontiguousarray(
            lg_c[:, :, :, 0:5].transpose(0, 1, 3, 2, 4).reshape(128, 5, 20, V))
        lgce_c = lgce_c.astype(ml_dtypes.float8_e3m4)

        x_c = xp[sl].transpose(0, 1, 3, 2).reshape(128, T * 20)
        wM_c = wMp[sl].transpose(0, 1, 3, 2).reshape(128, T * 20)
        tm_c = tmp[sl].reshape(128, 20)

        wm_c = np.concatenate([win_m[sl, :, 0], win_m[sl, :, 1]],
                              axis=0).reshape(128, NPAIR * WG)
        wi_c = np.concatenate([win_i[sl, :, 0], win_i[sl, :, 1]],
                              axis=0).reshape(128, NPAIR, WG)

        p2_c = np.empty((128, 512), dtype=np.float32)
        p2_c[:, 0:224] = wm_c
        p2_c[:, 224:344] = x_c
        p2_c[:, 344:464] = wM_c
        p2_c[:, 464:484] = tm_c
        p2_c[:, 484:512] = np.broadcast_to(aa[None, :], (128, V))

        in_maps.append({
            "lg5": lg5_c,
            "p2": p2_c,
            "lgce": lgce_c,
            "wini": wi_c.astype(ml_dtypes.bfloat16),
        })
    return in_maps


def _combine(results):
    ce_num = 0.0
    mf_cnt = 0.0
    sp_num = 0.0
    sp_cnt = 0.0
    for r in results:
        p = r["partials"].astype(np.float64)
        ce_num += p[:, 0].sum()
        mf_cnt += p[:, 1].sum()
        sp_num += p[:, 2].sum()
        sp_cnt += p[:, 3].sum()
    ce = ce_num / max(mf_cnt, 1.0)
    spec = 0.5 * sp_num / max(sp_cnt, 1.0)
    return np.float32(CE_W * ce + SPEC_W * spec)


def kernel(**inputs) -> np.ndarray:
    from concourse.bass_utils import run_bass_kernel_spmd

    nc = _get_nc()
    in_maps = _host_prep(**inputs)
    res = run_bass_kernel_spmd(nc, in_maps, core_ids=list(range(NCORES)))
    return _combine(res.results)
